# revision 16
# baseline (speedup 1.0000x reference)
"""Trainium2 Bass kernel for AvgReadout-style segment mean + L2 normalize.

reference:
    vsum[i] = sum over edges e with src[e]==i of emb[dst[e]]
    deg[i]  = count of such edges (clamped to >=1)
    out     = l2_normalize(vsum / deg, eps=1e-12)

Key identity: l2_normalize(vsum/deg) == l2_normalize(vsum) whenever deg >= 1
(positive per-row scalar doesn't change direction), and for deg == 0 both are
exactly 0.  So the kernel only needs vsum, never deg.

Distribution: edges are sorted by src on host and sharded by src-range across
8 cores (12500 segments each).  Each core's output slice is disjoint, so no
collectives are needed.

Per core the segments are processed in 98 blocks of 128, grouped into
superblocks of 4 blocks.  Edge rows are fetched with dma_gather (int16
indices, so emb is addressed as 4 quarter tables of 25000 rows); edges are
grouped into cells (block, quarter), padded to whole subtiles of 128 edges.
Cell capacities are maxed across cores so a single compiled program serves
all 8 cores.

Per superblock: one gather tile [128, NS*128] is filled by a handful of
dma_gather calls (<=2816 idx each, ring = dynamic_dma_scratch_size/16;
round-robined over 4 SWDGE queues; fewer/bigger calls matter: HW per-call
overhead ~2us dwarfs the per-descriptor cost), and ONE batched DVE compare builds all
subtile one-hots at once into an oh tile laid out c-major/t-minor
(oh[p, c*NSCAP + t] = (srcloc[p, slot0+t] == c)).  This layout keeps every
DVE operand's last AP dim packed (iota_rep is a host-materialized constant),
which hits the DVE 2x fast mode; subtile t's one-hot is then the strided
column slice oh[:, t::NSCAP], fed to the PE as lhsT with stride NSCAP.
Pad edges carry an out-of-range srcloc sentinel so their one-hot column is
all zeros.  Epilogue per block: sum-of-squares (ACT Square+accum), sqrt,
clamp 1e-12, reciprocal, scale-copy, DMA out.
"""

import numpy as np
from contextlib import ExitStack

N_SPOT = 100000
D = 128
P = 128
NCORES = 8
SEG_PER_CORE = 12500
NBLK = (SEG_PER_CORE + P - 1) // P  # 98
NQ = 4            # emb quarter tables (int16 index range)
QROWS = N_SPOT // NQ  # 25000
SB = 4            # blocks per superblock (PSUM tiles live concurrently)
CALL_CAP = 22     # subtiles per dma_gather call (<= 2816 idx ring)
NQUEUES = 4       # SWDGE queues to round-robin
DMA_SCRATCH = 45056  # bytes/partition of SWDGE ring (2816 descriptors/queue)
PAD_SENTINEL = 999.0


def compute_layout(capsub, cell_real=None):
    """capsub: [nblk, NQ] int array of per-cell subtile capacities.
    cell_real: optional [nblk*NQ] max real edge count per cell (across
    cores); used to trim trailing pad descriptors from each gather call.

    Returns dict with:
      nslots: total subtile slots
      slot_block: per-slot block id
      calls: list of (q, slot0, nsub, num_idxs) dma_gather calls, slot order
      blk_slots: per-block list of slot ids (ascending)
      sb_list: list of (blocks, (call_lo, call_hi), slot0, nslots_sb)
      nscap: max subtile slots in any superblock
    """
    capsub = np.asarray(capsub)
    nblk = capsub.shape[0]
    slot_block = []
    slot_real = []
    calls = []
    blk_slots = [[] for _ in range(nblk)]
    sb_list = []
    slot = 0
    nscap = 0
    for sb0 in range(0, nblk, SB):
        blocks = list(range(sb0, min(sb0 + SB, nblk)))
        call_lo = len(calls)
        sb_slot0 = slot
        for q in range(NQ):
            run = []  # slots of this (superblock, q) run
            for b in blocks:
                creal = (
                    int(cell_real[b * NQ + q]) if cell_real is not None
                    else int(capsub[b, q]) * P
                )
                for j in range(int(capsub[b, q])):
                    slot_block.append(b)
                    slot_real.append(max(0, min(P, creal - j * P)))
                    blk_slots[b].append(slot)
                    run.append(slot)
                    slot += 1
            for i in range(0, len(run), CALL_CAP):
                chunk = run[i : i + CALL_CAP]
                # trim trailing all-pad subtiles; keep >=1 slot
                keep = len(chunk)
                while keep > 1 and slot_real[chunk[keep - 1]] == 0:
                    keep -= 1
                num_idxs = (keep - 1) * P + max(1, slot_real[chunk[keep - 1]])
                calls.append((q, chunk[0], len(chunk), num_idxs))
        sb_list.append((blocks, (call_lo, len(calls)), sb_slot0, slot - sb_slot0))
        nscap = max(nscap, slot - sb_slot0)
    return {
        "nslots": slot,
        "slot_block": slot_block,
        "calls": calls,
        "blk_slots": blk_slots,
        "sb_list": sb_list,
        "nscap": nscap,
    }


def preprocess(emb, mask, ncores=NCORES, seg_per_core=SEG_PER_CORE, nblk=NBLK,
               nspot=N_SPOT):
    """Sort/shard/pad edges. Returns (in_maps, capsub, layout)."""
    qrows = nspot // NQ
    emb = np.ascontiguousarray(np.asarray(emb, dtype=np.float32))
    emb16 = emb.astype(np.float16)
    mask = np.asarray(mask)
    src = mask[0].astype(np.int64, copy=False)
    dst = mask[1].astype(np.int64, copy=False)

    order = np.argsort(src, kind="stable")
    src_s = src[order].astype(np.int32)
    dst_s = dst[order].astype(np.int32)

    core_bounds = np.searchsorted(
        src_s, (seg_per_core * np.arange(ncores + 1)).astype(np.int32)
    )

    percore = []
    cnts = np.zeros((ncores, nblk * NQ), np.int64)
    for k in range(ncores):
        lo, hi = int(core_bounds[k]), int(core_bounds[k + 1])
        s = src_s[lo:hi] - seg_per_core * k
        d = dst_s[lo:hi]
        cell = (s >> 7) * NQ + d // qrows
        o = np.lexsort((d, cell))
        s, d, cell = s[o], d[o], cell[o]
        cnts[k] = np.bincount(cell, minlength=nblk * NQ)
        percore.append((s, d, cell))

    capsub = (-(-cnts.max(axis=0) // P)).reshape(nblk, NQ).astype(np.int64)
    layout = compute_layout(capsub)
    nslots = layout["nslots"]
    nscap = layout["nscap"]

    # slot base per cell, following the layout's slot order
    cell_slot0 = np.zeros(nblk * NQ, np.int64)
    slot = 0
    for sb0 in range(0, nblk, SB):
        for q in range(NQ):
            for b in range(sb0, min(sb0 + SB, nblk)):
                cell_slot0[b * NQ + q] = slot
                slot += int(capsub[b, q])
    assert slot == nslots

    # iota_rep[p, c*nscap + t] = c, for the c-major one-hot compare
    iota_rep = np.repeat(np.arange(P, dtype=np.float16), nscap)
    iota_rep = np.broadcast_to(iota_rep[None, :], (P, P * nscap)).copy()

    in_maps = []
    for k in range(ncores):
        s, d, cell = percore[k]
        cum = np.zeros(nblk * NQ, np.int64)
        cc = cnts[k]
        cum[1:] = np.cumsum(cc)[:-1]
        rank = np.arange(len(s), dtype=np.int64) - cum[cell]
        pos = cell_slot0[cell] * P + rank  # global edge position

        # srcloc padded by nscap so the batched compare can over-read
        srcloc = np.full((nslots + nscap) * P, PAD_SENTINEL, np.float16)
        srcloc[pos] = (s & 127).astype(np.float16)
        dloc = np.zeros(nslots * P, np.int16)
        dloc[pos] = (d % qrows).astype(np.int16)

        # srcloc tile [p, slot] = value of edge (slot, p)
        srcloc_t = np.ascontiguousarray(srcloc.reshape(nslots + nscap, P).T)
        # idx16 [j%16, slot*8 + j//16] = dloc of edge (slot, j), replicated
        # across the 8 partition groups for the Q7 ucode.
        idx_blk = np.ascontiguousarray(
            dloc.reshape(nslots * 8, 16).T
        )  # [16, nslots*8]
        idx16 = np.tile(idx_blk, (8, 1))
        in_maps.append(
            {"emb": emb16, "srcloc": srcloc_t, "dstidx": idx16,
             "iota_rep": iota_rep}
        )
    return in_maps, capsub, layout


def build_program(capsub, layout, nblk=NBLK, nspot=N_SPOT, d=D, repeats=1,
                  hw_loop=False):
    import concourse.bass as bass
    import concourse.tile as tile
    from concourse import bacc, mybir

    qrows = nspot // NQ
    nslots = layout["nslots"]
    calls = layout["calls"]
    blk_slots = layout["blk_slots"]
    sb_list = layout["sb_list"]
    nscap = layout["nscap"]

    nc = bacc.Bacc(
        "TRN2", target_bir_lowering=False, debug=False,
        num_swdge_queues=NQUEUES, dynamic_dma_scratch_size=DMA_SCRATCH,
    )
    emb_t = nc.dram_tensor("emb", [nspot, d], mybir.dt.float16, kind="ExternalInput")
    srcloc_t = nc.dram_tensor(
        "srcloc", [P, nslots + nscap], mybir.dt.float16, kind="ExternalInput"
    )
    dstidx_t = nc.dram_tensor(
        "dstidx", [P, nslots * 8], mybir.dt.int16, kind="ExternalInput"
    )
    iota_t = nc.dram_tensor(
        "iota_rep", [P, P * nscap], mybir.dt.float16, kind="ExternalInput"
    )
    # fp16 output halves the out-DMA traffic; values are L2-normalized
    # (|x| <= 1) so fp16 costs ~5e-4 rel err vs the 2e-2 gate. The host
    # upcasts to float32.
    out_t = nc.dram_tensor(
        "out", [nblk * P, d], mybir.dt.float16, kind="ExternalOutput"
    )

    with tile.TileContext(nc) as tc, ExitStack() as ctx:
        consts = ctx.enter_context(tc.tile_pool(name="consts", bufs=1))
        gpool = ctx.enter_context(tc.tile_pool(name="gather", bufs=3))
        ohpool = ctx.enter_context(tc.tile_pool(name="onehot", bufs=2))
        spool = ctx.enter_context(tc.tile_pool(name="scratch", bufs=4))
        opool = ctx.enter_context(tc.tile_pool(name="outs", bufs=4))
        ppool = ctx.enter_context(tc.tile_pool(name="psum", bufs=8, space="PSUM"))

        srcloc_sb = consts.tile([P, nslots + nscap], mybir.dt.float16)
        dstidx_sb = consts.tile([P, nslots * 8], mybir.dt.int16)
        iota_sb = consts.tile([P, P * nscap], mybir.dt.float16)

        out_ap = out_t.ap()
        emb_ap = emb_t.ap()

        def body():
            nc.sync.dma_start(srcloc_sb[:], srcloc_t.ap())
            nc.sync.dma_start(iota_sb[:], iota_t.ap())
            # dstidx is loaded in per-superblock chunks so the first gather
            # call starts after ~0.5us instead of waiting for the full 4MB
            # index table (~12us) on the execution critical path.
            dstidx_ap = dstidx_t.ap()
            callno = 0
            for blocks, (clo, chi), sb_slot0, ns_sb in sb_list:
                nc.sync.dma_start(
                    dstidx_sb[:, sb_slot0 * 8 : (sb_slot0 + ns_sb) * 8],
                    dstidx_ap[:, sb_slot0 * 8 : (sb_slot0 + ns_sb) * 8],
                )
                gt = gpool.tile([P, nscap * d], mybir.dt.float16, tag="gt")
                for ci in range(clo, chi):
                    q, s0, nsub, num_idxs = calls[ci]
                    # num_idxs == nsub*P: trailing-pad trimming is disabled
                    # because skipped descriptors would leave uninitialized
                    # SBUF in gt, and 0*NaN garbage would poison the PSUM.
                    u0 = s0 - sb_slot0
                    nc.gpsimd.dma_gather(
                        out_ap=gt[:, u0 * d : (u0 + nsub) * d].rearrange(
                            "p (c e) -> p c e", e=d
                        ),
                        in_ap=emb_ap[q * qrows : (q + 1) * qrows, :],
                        idxs_ap=dstidx_sb[:, s0 * 8 : (s0 + nsub) * 8],
                        num_idxs=num_idxs,
                        num_idxs_reg=num_idxs,
                        elem_size=d,
                        single_packet=False,
                        queue_num=callno % NQUEUES,
                    )
                    callno += 1
                # one batched one-hot build for the whole superblock:
                # oh[p, c*ns_sb + u] = (iota_rep[p, c*nscap+u] ==
                #                       srcloc[p, sb_slot0+u])  for u < ns_sb
                # (iota_rep is laid out at stride nscap; reading it with a
                # [nscap, P] x [1, ns_sb] AP yields the same c values at the
                # packed ns_sb-stride output layout.)
                oh = ohpool.tile([P, P * nscap], mybir.dt.float16, tag="oh")
                srl = srcloc_sb[:, sb_slot0 : sb_slot0 + ns_sb]
                srl_b = bass.AP(
                    srl.tensor, srl.offset, [srl.ap[0], [0, P], [1, ns_sb]]
                )
                iot = iota_sb[:, : (P - 1) * nscap + ns_sb]
                iot_b = bass.AP(
                    iot.tensor, iot.offset, [iot.ap[0], [nscap, P], [1, ns_sb]]
                )
                oh_o = oh[:, : P * ns_sb]
                oh_b = bass.AP(
                    oh_o.tensor, oh_o.offset, [oh_o.ap[0], [ns_sb, P], [1, ns_sb]]
                )
                nc.vector.tensor_tensor(
                    out=oh_b,
                    in0=iot_b,
                    in1=srl_b,
                    op=mybir.AluOpType.is_equal,
                )
                for b in blocks:
                    slots = blk_slots[b]
                    if not slots:
                        ot = opool.tile([P, d], mybir.dt.float16)
                        nc.vector.memset(ot[:], 0.0)
                        nc.sync.dma_start(out_ap[b * P : (b + 1) * P, :], ot[:])
                        continue
                    ps = ppool.tile([P, d], mybir.dt.float32, space="PSUM")
                    for i, sl in enumerate(slots):
                        u = sl - sb_slot0
                        lsrc = oh[:, u : u + (P - 1) * ns_sb + 1]
                        lhsT = bass.AP(
                            lsrc.tensor, lsrc.offset, [lsrc.ap[0], [ns_sb, P]]
                        )
                        nc.tensor.matmul(
                            ps[:],
                            lhsT=lhsT,
                            rhs=gt[:, u * d : (u + 1) * d],
                            start=(i == 0),
                            stop=(i == len(slots) - 1),
                        )
                    sq = spool.tile([P, d], mybir.dt.float32)
                    ss = spool.tile([P, 1], mybir.dt.float32)
                    nc.scalar.activation(
                        sq[:],
                        ps[:],
                        mybir.ActivationFunctionType.Square,
                        accum_out=ss[:],
                    )
                    nrm = spool.tile([P, 1], mybir.dt.float32)
                    nc.scalar.activation(
                        nrm[:], ss[:], mybir.ActivationFunctionType.Sqrt
                    )
                    nc.vector.tensor_scalar(
                        out=nrm[:],
                        in0=nrm[:],
                        scalar1=1e-12,
                        scalar2=None,
                        op0=mybir.AluOpType.max,
                    )
                    nc.vector.reciprocal(nrm[:], nrm[:])
                    ot = opool.tile([P, d], mybir.dt.float16)
                    nc.scalar.activation(
                        ot[:],
                        ps[:],
                        mybir.ActivationFunctionType.Copy,
                        scale=nrm[:],
                    )
                    nc.sync.dma_start(out_ap[b * P : (b + 1) * P, :], ot[:])

        if hw_loop and repeats > 1:
            with tc.For_i(0, repeats):
                body()
        else:
            for _ in range(repeats):
                body()

    nc.compile()
    return nc


_PROGRAM_CACHE = {}


def _get_program(capsub, layout):
    key = capsub.tobytes()
    if key not in _PROGRAM_CACHE:
        _PROGRAM_CACHE[key] = build_program(capsub, layout)
    return _PROGRAM_CACHE[key]


def kernel(**inputs):
    emb = inputs["emb"]
    mask = inputs["mask"]
    in_maps, capsub, layout = preprocess(emb, mask)
    nc = _get_program(capsub, layout)

    import time
    from concourse.bass_utils import run_bass_kernel_spmd

    res = None
    err = None
    for attempt in range(3):
        try:
            res = run_bass_kernel_spmd(nc, in_maps, core_ids=list(range(NCORES)))
            break
        except Exception as e:  # noqa: BLE001 - transient axon UNAVAILABLE
            err = e
            time.sleep(3)
    if res is None:
        raise err
    out = np.empty((N_SPOT, D), np.float32)
    for k in range(NCORES):
        out[k * SEG_PER_CORE : (k + 1) * SEG_PER_CORE] = res.results[k]["out"][
            :SEG_PER_CORE
        ]  # fp16 -> fp32 upcast on assignment
    return out
